# revision 20
# baseline (speedup 1.0000x reference)
"""CrossMerge kernel for trn2 — v5b (fp16 merge + fp32 gpsimd broadcast).

Math (per batch element):
    means_i = mean over C of g_i              (4, H, W)
    logits  = w_proj @ means + b_proj         (4, H, W)
    w       = softmax(logits, axis=0)         (4, H, W)
    out     = sum_i g_i * w_i                 (C, H, W)

Sharding: data-parallel over batch B=8 across 8 cores; weights replicated;
no cross-device communication.

Key design points (v4 measured 157.5us, v3 fp32 baseline 247us):
 - Grids are converted to fp16 on HOST; output returned fp16 and
   upconverted on host.  Kernel HBM traffic 23.7MB/core (was 47.3).
 - All merge elementwise work runs on DVE in fp16: every operand
   2-byte/packed/SBUF engages the DVE 2x_1p perf mode (HW-verified in v4:
   [128,2304] tensor_tensor = 1352ns = 0.52 ns/col).
 - v4's bottleneck was PE (148us busy: logits + 20 bcast matmuls/d-tile
   + LDWEIGHTS per matmul) and ACT (104us: 80 staging copies + sems).
   v5 moves the weight broadcast to the otherwise-idle gpsimd
   (partition_broadcast, attn library).  gpsimd Q7 ucode hangs the exec
   unit on ANY 2-byte operand (HW: v3 fp16 tensor ops, v5a uint16
   broadcast -> NRT_EXEC_UNIT_UNRECOVERABLE), so the broadcast runs in
   fp32 and the otherwise-idle ACT engine downcasts wqf32 -> wq fp16
   (0.833 ns/col, 4 passes).
 - The BIR verifier requires gpsimd ISA src APs to start at partition 0,
   so a tiny SBUF->SBUF DMA packs W4's four rows [4,W] into a partition-0
   tile W4row [1,4,W]; the broadcasts read base-0 slices of it.
 - PE keeps only logits (8 accumulating matmuls/jslice: contraction=128
   channels, accumulated over 4 grids x 2 chunks) + S4 denominator.
   Logits use the exp-scale trick: lhsT holds w[o,i] raw (fp16-safe
   O(0.1)); the /C lands in the exp activation scale.
 - E/S4/recip/W4 are d-tile-wide [4,1536] (one recip + one W4 mul per
   d-tile); S4's three 512-col denominator matmuls land in one 3-bank
   PSUM tile (each matmul writes within a single bank).
 - Emission order per iter d (what matters is each engine's queue order):
     dma_in(d+1) | gpsimd bcast(d-1) + ACT convert(d-1) | DVE
     products/adds(d-1) + dma_out(d-1) | narrow(d): PE logits, ACT exp,
     PE S4, DVE recip/W4 | dma W4row(d)
   Wide-before-narrow on DVE keeps the DVE busy on products(d-1) while
   dma_in(d) lands; W4(d) still completes in time for bcast(d) at the
   top of iter d+1.
"""

import os
import sys

import numpy as np

try:
    import concourse.bass as bass
except ImportError:  # fresh grading dir: concourse lives in the container repo
    sys.path.insert(0, "/opt/trn_rl_repo")
    import concourse.bass as bass

from contextlib import ExitStack

import concourse.tile as tile
from concourse import bacc, library_config, mybir
from concourse.bass_utils import run_bass_kernel_spmd

B, C, H, W = 8, 256, 96, 96
HW = H * W  # 9216
NCORES = 8
CPB = C // 128  # 2 partition chunks per core
MAXW = 1536  # d-tile width; 6 uniform tiles of 3 jslices
ND = HW // MAXW  # 6
NJ = MAXW // 512  # 3

F32 = mybir.dt.float32
F16 = mybir.dt.float16
AF = mybir.ActivationFunctionType

_CACHE = {}


def build_program():
    nc = bacc.Bacc("TRN2", debug=False, num_devices=NCORES)

    gall_d = nc.dram_tensor("gall", [4, C, HW], F16, kind="ExternalInput").ap()
    # fp16 constants: cols 0-15 ws (w[o,i] at col 4i+o, replicated down
    # partitions), cols 16-19 ones4x4
    ch_d = nc.dram_tensor("cblob16", [128, 20], F16, kind="ExternalInput").ap()
    # fp32 constants: col 0 rows 0-3 = b_proj
    cb_d = nc.dram_tensor("cblob", [128, 1], F32, kind="ExternalInput").ap()
    out = nc.dram_tensor("out", [C, HW], F16, kind="ExternalOutput").ap()

    with tile.TileContext(nc) as tc, ExitStack() as ctx:
        const = ctx.enter_context(tc.tile_pool(name="const", bufs=1))
        gin = ctx.enter_context(tc.tile_pool(name="gin", bufs=3))
        outp = ctx.enter_context(tc.tile_pool(name="outp", bufs=2))
        narrow = ctx.enter_context(tc.tile_pool(name="narrow", bufs=3))
        wqp = ctx.enter_context(tc.tile_pool(name="wqp", bufs=2))
        wfp = ctx.enter_context(tc.tile_pool(name="wfp", bufs=1))
        prod = ctx.enter_context(tc.tile_pool(name="prod", bufs=3))
        qpool = ctx.enter_context(tc.tile_pool(name="qpool", bufs=3))
        ps_nar = ctx.enter_context(tc.tile_pool(name="psnar", bufs=2, space="PSUM"))

        ch = const.tile([128, 20], F16)
        nc.sync.dma_start(out=ch[:], in_=ch_d)
        cb = const.tile([128, 1], F32)
        nc.sync.dma_start(out=cb[:], in_=cb_d)
        bv = cb[0:4, 0:1]

        # partition_broadcast lives in the 'attn' gpsimd library; load once.
        nc.gpsimd.load_library(library_config.attn)

        def ws_i(i):  # [128, 4] logits lhsT for grid i
            return ch[:, 4 * i : 4 * i + 4]

        ones4 = ch[0:4, 16:20]

        # Warmup matmul: absorbs the const-blob DMA wait on the PE clock.
        warm = ps_nar.tile([128, 512], F32, tag="smx")
        nc.tensor.matmul(warm[0:4, 0:16], lhsT=ch[0:4, 0:4], rhs=ch[0:4, 0:16],
                         start=True, stop=True)

        def narrow_stage(d, gat):
            """Softmax chain: logits (jslice pairs) -> exp -> S4 -> one
            d-tile-wide recip + W4.  Pairing hides the exp->S4 PE<->ACT
            round trip behind the partner slice's matmuls.  Matmul outputs
            at PSUM base partition 0 (ISA constraint)."""
            jslc = [(x0, 512) for x0 in range(0, MAXW, 512)]
            Ed = narrow.tile([4, MAXW], F16, tag="E", bufs=2)
            S4d = ps_nar.tile([4, MAXW], F32, tag="S4", bufs=1)
            for pair in [jslc[k : k + 2] for k in range(0, len(jslc), 2)]:
                Ls = []
                for x0, n in pair:
                    L = ps_nar.tile([128, 512], F32, tag="smx")
                    Ls.append(L[0:4, 0:n])
                    k = 0
                    for i in range(4):
                        for c in range(CPB):
                            nc.tensor.matmul(
                                Ls[-1],
                                lhsT=ws_i(i),
                                rhs=gat[:, i, c, x0 : x0 + n],
                                start=(k == 0),
                                stop=(k == 7),
                            )
                            k += 1
                for pi, (x0, n) in enumerate(pair):
                    nc.scalar.activation(Ed[0:4, x0 : x0 + n], Ls[pi], AF.Exp,
                                         bias=bv, scale=1.0 / C)
                for pi, (x0, n) in enumerate(pair):
                    # each matmul writes within a single PSUM bank
                    nc.tensor.matmul(S4d[0:4, x0 : x0 + n], lhsT=ones4,
                                     rhs=Ed[0:4, x0 : x0 + n],
                                     start=True, stop=True)
            R4 = narrow.tile([4, MAXW], F32, tag="R4", bufs=2)
            nc.vector.reciprocal_approx_fast(R4[:], S4d[:])
            W4d = narrow.tile([4, MAXW], F32, tag="W4", bufs=2)
            nc.vector.tensor_mul(W4d[:], Ed[:], R4[:])
            # pack the 4 rows into partition 0 so the gpsimd broadcast can
            # read base-0 slices (BIR verifier rejects src base partition>0)
            W4row = narrow.tile([1, 4, MAXW], F32, tag="W4row", bufs=1)
            nc.sync.dma_start(out=W4row[:], in_=W4d[:])
            return W4row

        def bcast_stage(prev):
            """gpsimd partition_broadcast of packed W4 rows to 128 partitions
            (fp32: Q7 ucode hangs on 2-byte operands) + ACT downcast to the
            fp16 wq tiles the DVE products read."""
            if prev is None:
                return None
            d, gat, ot, W4row = prev
            wq = {}
            for i in range(4):
                wf = wfp.tile([128, MAXW], F32, tag=f"wf{i}")
                nc.gpsimd.partition_broadcast(
                    wf[:], W4row[0:1, i, :], channels=128
                )
                wqt = wqp.tile([128, MAXW], F16, tag=f"wq{i}")
                wq[i] = wqt
                nc.scalar.copy(wqt[:], wf[:])
            return (d, gat, ot, wq)

        def wide_stage(staged):
            """DVE products + add tree + store for iter d-1."""
            if staged is None:
                return
            d, gat, ot, wq = staged
            for c in range(CPB):
                p = {}
                for i in range(4):
                    pt = prod.tile([128, MAXW], F16, tag="p")
                    nc.vector.tensor_mul(pt[:], gat[:, i, c, :], wq[i][:])
                    p[i] = pt
                    if i == 1:
                        q01 = qpool.tile([128, MAXW], F16, tag="q")
                        nc.vector.tensor_add(q01[:], p[0][:], p[1][:])
                q23 = qpool.tile([128, MAXW], F16, tag="q")
                nc.vector.tensor_add(q23[:], p[2][:], p[3][:])
                nc.vector.tensor_add(ot[:, c, :], q01[:], q23[:])
            n0 = d * MAXW
            nc.sync.dma_start(
                out=out[:, n0 : n0 + MAXW].rearrange("(c p) n -> p c n", c=CPB),
                in_=ot[:],
            )

        def dma_in(d):
            n0 = d * MAXW
            gat = gin.tile([128, 4, CPB, MAXW], F16, tag="gall")
            nc.sync.dma_start(
                out=gat[:],
                in_=gall_d[:, :, n0 : n0 + MAXW].rearrange(
                    "i (c p) n -> p i c n", c=CPB
                ),
            )
            return gat

        gats = {0: dma_in(0)}
        prev = None  # (d, gat, ot, W4row) awaiting bcast+wide
        for d in range(ND):
            if d + 1 < ND:
                gats[d + 1] = dma_in(d + 1)
            staged = bcast_stage(prev)
            wide_stage(staged)
            gat = gats.pop(d)
            ot = outp.tile([128, CPB, MAXW], F16, tag="ot")
            W4row = narrow_stage(d, gat)
            prev = (d, gat, ot, W4row)
        wide_stage(bcast_stage(prev))

    nc.compile()
    return nc


def _get_program():
    if "nc" not in _CACHE:
        _CACHE["nc"] = build_program()
    return _CACHE["nc"]


def make_cblobs(w_proj, b_proj):
    w = np.asarray(w_proj, dtype=np.float32)
    b = np.asarray(b_proj, dtype=np.float32)
    ch = np.zeros((128, 20), dtype=np.float16)
    for i in range(4):
        for o in range(4):
            ch[:, 4 * i + o] = np.float16(w[o, i])
    ch[0:4, 16:20] = 1.0
    cb = np.zeros((128, 1), dtype=np.float32)
    cb[0:4, 0] = b
    return ch, cb


LAST_RESULT = None


def kernel(g0, g1, g2, g3, w_proj, b_proj):
    global LAST_RESULT
    nc = _get_program()

    ch, cb = make_cblobs(w_proj, b_proj)

    gall = np.stack(
        [np.asarray(x).reshape(B, C, HW).astype(np.float16) for x in (g0, g1, g2, g3)],
        axis=1,
    )  # (B, 4, C, HW) fp16
    in_maps = []
    for bi in range(NCORES):
        m = {"gall": np.ascontiguousarray(gall[bi]), "cblob16": ch, "cblob": cb}
        in_maps.append(m)

    res = run_bass_kernel_spmd(
        nc,
        in_maps,
        list(range(NCORES)),
        trace=bool(int(os.environ.get("CM_TRACE", "0"))),
        tmpdir=os.environ.get("CM_TRACE_DIR") or None,
    )
    LAST_RESULT = res
    out_full = np.stack(
        [
            res.results[bi]["out"].astype(np.float32).reshape(C, H, W)
            for bi in range(NCORES)
        ],
        axis=0,
    )
    return out_full


# revision 21
# speedup vs baseline: 1.0196x; 1.0196x over previous
"""CrossMerge kernel for trn2 — v6 (fp16 merge + hybrid gpsimd/PE broadcast).

Math (per batch element):
    means_i = mean over C of g_i              (4, H, W)
    logits  = w_proj @ means + b_proj         (4, H, W)
    w       = softmax(logits, axis=0)         (4, H, W)
    out     = sum_i g_i * w_i                 (C, H, W)

Sharding: data-parallel over batch B=8 across 8 cores; weights replicated;
no cross-device communication.

Measured history: v3 fp32 247us; v4 fp16 157.5us (PE-bound: 148us busy =
logits + 4 bcast passes + LDWEIGHTS); v5b all-gpsimd-bcast 185.7us (PE
fine at 93 but DVE inflated to 154 by SBUF port contention from 38MB of
fp32 broadcast traffic + longer per-iter sem chains).

v6 design:
 - Grids fp16 on HOST (HBM traffic 23.7MB/core); output fp16, host
   upconverts.  Merge (8 products + 6 adds per d-tile) on DVE, all
   operands fp16/packed/SBUF -> 2x_1p mode, 0.52 ns/col (HW-verified).
 - PE runs at ~0.85 ns/output-col at the observed (throttled) clock, and
   walrus emits LDWEIGHTS per matmul (~93ns), so PE cost is simply
   passes x cols.  PE keeps: logits 8 passes (irreducible: contraction
   over 128 channels x (4 grids x 2 chunks), exp-scale trick folds /C
   into the exp), S4 denominator 1 pass, and ONE broadcast pass (grid 3).
 - Grids 0-2 broadcast on the otherwise-idle gpsimd via
   partition_broadcast (attn library).  Q7 ucode hangs on ANY 2-byte
   operand (v3 fp16 tensor ops, v5a uint16 -> NRT_EXEC_UNIT_UNRECOVERABLE)
   so those run fp32 from a partition-0 packed W4row (BIR verifier
   rejects gpsimd ISA src APs at base partition > 0; a tiny SBUF->SBUF
   DMA packs W4d rows 0-2), then ACT downcasts to the fp16 wq tiles.
 - ACT load: 3 downcasts + grid-3 PSUM staging + exp + W4h copy ~ 60us.
 - Narrow tail is d-tile-wide: one recip [4,1536] and one W4 mul per
   d-tile (not per jslice); S4's three 512-col denominator matmuls land
   in one 3-bank PSUM tile (each matmul writes within a single bank).
 - Per-iter emission (engine queue order is what matters):
     dma_in(d+1) | bcast(d-1): gpsimd pb x3 + PE bcast3 + ACT converts/
     stage | DVE products/adds(d-1) + dma_out(d-1) | narrow(d) | pack
     W4row(d).
   Wide-before-narrow on DVE keeps DVE busy while dma_in(d) lands; the
   products for grids 0,1 start as soon as their converts finish.
"""

import os
import sys

import numpy as np

try:
    import concourse.bass as bass
except ImportError:  # fresh grading dir: concourse lives in the container repo
    sys.path.insert(0, "/opt/trn_rl_repo")
    import concourse.bass as bass

from contextlib import ExitStack

import concourse.tile as tile
from concourse import bacc, library_config, mybir
from concourse.bass_utils import run_bass_kernel_spmd

B, C, H, W = 8, 256, 96, 96
HW = H * W  # 9216
NCORES = 8
CPB = C // 128  # 2 partition chunks per core
MAXW = 1536  # d-tile width; 6 uniform tiles of 3 jslices
ND = HW // MAXW  # 6
NJ = MAXW // 512  # 3
NGP = 3  # grids broadcast on gpsimd (0..NGP-1); the rest use PE+ACT

F32 = mybir.dt.float32
F16 = mybir.dt.float16
AF = mybir.ActivationFunctionType

_CACHE = {}


def build_program():
    nc = bacc.Bacc("TRN2", debug=False, num_devices=NCORES)

    gall_d = nc.dram_tensor("gall", [4, C, HW], F16, kind="ExternalInput").ap()
    # fp16 constants: cols 0-511 sel (one-hot bcast lhsT rows 0-3), cols
    # 512-527 ws (w[o,i] at col 512+4i+o, replicated down partitions),
    # cols 528-531 ones4x4
    ch_d = nc.dram_tensor("cblob16", [128, 532], F16, kind="ExternalInput").ap()
    # fp32 constants: col 0 rows 0-3 = b_proj
    cb_d = nc.dram_tensor("cblob", [128, 1], F32, kind="ExternalInput").ap()
    out = nc.dram_tensor("out", [C, HW], F16, kind="ExternalOutput").ap()

    with tile.TileContext(nc) as tc, ExitStack() as ctx:
        const = ctx.enter_context(tc.tile_pool(name="const", bufs=1))
        gin = ctx.enter_context(tc.tile_pool(name="gin", bufs=3))
        outp = ctx.enter_context(tc.tile_pool(name="outp", bufs=2))
        narrow = ctx.enter_context(tc.tile_pool(name="narrow", bufs=2))
        wqp = ctx.enter_context(tc.tile_pool(name="wqp", bufs=2))
        wfp = ctx.enter_context(tc.tile_pool(name="wfp", bufs=1))
        prod = ctx.enter_context(tc.tile_pool(name="prod", bufs=3))
        qpool = ctx.enter_context(tc.tile_pool(name="qpool", bufs=3))
        ps_nar = ctx.enter_context(tc.tile_pool(name="psnar", bufs=2, space="PSUM"))
        ps_wb = ctx.enter_context(tc.tile_pool(name="pswb", bufs=2, space="PSUM"))

        ch = const.tile([128, 532], F16)
        nc.sync.dma_start(out=ch[:], in_=ch_d)
        cb = const.tile([128, 1], F32)
        nc.sync.dma_start(out=cb[:], in_=cb_d)
        bv = cb[0:4, 0:1]

        # partition_broadcast lives in the 'attn' gpsimd library; load once.
        nc.gpsimd.load_library(library_config.attn)

        def ws_i(i):  # [128, 4] logits lhsT for grid i
            return ch[:, 512 + 4 * i : 512 + 4 * i + 4]

        def sel_i(i):  # [4, 128] bcast lhsT for grid i
            return ch[0:4, 128 * i : 128 * (i + 1)]

        ones4 = ch[0:4, 528:532]

        # Warmup matmul: absorbs the const-blob DMA wait on the PE clock.
        warm = ps_nar.tile([128, 512], F32, tag="smx")
        nc.tensor.matmul(warm[0:4, 0:16], lhsT=ch[0:4, 0:4], rhs=ch[0:4, 0:16],
                         start=True, stop=True)

        def narrow_stage(d, gat):
            """Softmax chain: logits (jslice pairs) -> exp -> S4 -> one
            d-tile-wide recip + W4 (fp32) + fp16 copy for the PE bcast rhs.
            Matmul outputs at PSUM base partition 0 (ISA constraint)."""
            jslc = [(x0, 512) for x0 in range(0, MAXW, 512)]
            Ed = narrow.tile([4, MAXW], F16, tag="E", bufs=2)
            S4d = ps_nar.tile([4, MAXW], F32, tag="S4", bufs=1)
            for pair in [jslc[k : k + 2] for k in range(0, len(jslc), 2)]:
                Ls = []
                for x0, n in pair:
                    L = ps_nar.tile([128, 512], F32, tag="smx")
                    Ls.append(L[0:4, 0:n])
                    k = 0
                    for i in range(4):
                        for c in range(CPB):
                            nc.tensor.matmul(
                                Ls[-1],
                                lhsT=ws_i(i),
                                rhs=gat[:, i, c, x0 : x0 + n],
                                start=(k == 0),
                                stop=(k == 7),
                            )
                            k += 1
                for pi, (x0, n) in enumerate(pair):
                    nc.scalar.activation(Ed[0:4, x0 : x0 + n], Ls[pi], AF.Exp,
                                         bias=bv, scale=1.0 / C)
                for pi, (x0, n) in enumerate(pair):
                    # each matmul writes within a single PSUM bank
                    nc.tensor.matmul(S4d[0:4, x0 : x0 + n], lhsT=ones4,
                                     rhs=Ed[0:4, x0 : x0 + n],
                                     start=True, stop=True)
            R4 = narrow.tile([4, MAXW], F32, tag="R4", bufs=1)
            nc.vector.reciprocal_approx_fast(R4[:], S4d[:])
            W4d = narrow.tile([4, MAXW], F32, tag="W4", bufs=1)
            nc.vector.tensor_mul(W4d[:], Ed[:], R4[:])
            # fp16 copy: PE bcast rhs must be fp16 (fp32 rhs = 4 cyc/row)
            W4h = narrow.tile([4, MAXW], F16, tag="W4h", bufs=2)
            nc.scalar.copy(W4h[:], W4d[:])
            # pack gpsimd grids' rows to partition 0 (BIR verifier rejects
            # gpsimd ISA src APs with base partition > 0)
            W4row = narrow.tile([1, NGP, MAXW], F32, tag="W4row", bufs=1)
            nc.sync.dma_start(out=W4row[:], in_=W4d[0:NGP, :])
            return (W4row, W4h)

        def bcast_stage(prev):
            """Weight broadcast for iter d-1: grids 0..NGP-1 via gpsimd
            partition_broadcast (fp32; Q7 hangs on 2-byte operands) + ACT
            downcast; remaining grids via PE matmul + ACT PSUM staging."""
            if prev is None:
                return None
            d, gat, ot, W4row, W4h = prev
            wq = {}
            for i in range(4):
                wqt = wqp.tile([128, MAXW], F16, tag=f"wq{i}")
                wq[i] = wqt
            for i in range(NGP):
                wf = wfp.tile([128, MAXW], F32, tag=f"wf{i}")
                nc.gpsimd.partition_broadcast(
                    wf[:], W4row[0:1, i, :], channels=128
                )
                nc.scalar.copy(wq[i][:], wf[:])
            for i in range(NGP, 4):
                for x0 in range(0, MAXW, 512):
                    Wb = ps_wb.tile([128, 512], F32, tag="wb")
                    nc.tensor.matmul(Wb[:], lhsT=sel_i(i),
                                     rhs=W4h[0:4, x0 : x0 + 512],
                                     start=True, stop=True)
                    nc.scalar.copy(wq[i][:, x0 : x0 + 512], Wb[:])
            return (d, gat, ot, wq)

        def wide_stage(staged):
            """DVE products + add tree + store for iter d-1."""
            if staged is None:
                return
            d, gat, ot, wq = staged
            for c in range(CPB):
                p = {}
                for i in range(4):
                    pt = prod.tile([128, MAXW], F16, tag="p")
                    nc.vector.tensor_mul(pt[:], gat[:, i, c, :], wq[i][:])
                    p[i] = pt
                    if i == 1:
                        q01 = qpool.tile([128, MAXW], F16, tag="q")
                        nc.vector.tensor_add(q01[:], p[0][:], p[1][:])
                q23 = qpool.tile([128, MAXW], F16, tag="q")
                nc.vector.tensor_add(q23[:], p[2][:], p[3][:])
                nc.vector.tensor_add(ot[:, c, :], q01[:], q23[:])
            n0 = d * MAXW
            nc.sync.dma_start(
                out=out[:, n0 : n0 + MAXW].rearrange("(c p) n -> p c n", c=CPB),
                in_=ot[:],
            )

        def dma_in(d):
            n0 = d * MAXW
            gat = gin.tile([128, 4, CPB, MAXW], F16, tag="gall")
            nc.sync.dma_start(
                out=gat[:],
                in_=gall_d[:, :, n0 : n0 + MAXW].rearrange(
                    "i (c p) n -> p i c n", c=CPB
                ),
            )
            return gat

        gats = {0: dma_in(0)}
        prev = None  # (d, gat, ot, W4row, W4h) awaiting bcast+wide
        for d in range(ND):
            if d + 1 < ND:
                gats[d + 1] = dma_in(d + 1)
            staged = bcast_stage(prev)
            wide_stage(staged)
            gat = gats.pop(d)
            ot = outp.tile([128, CPB, MAXW], F16, tag="ot")
            W4row, W4h = narrow_stage(d, gat)
            prev = (d, gat, ot, W4row, W4h)
        wide_stage(bcast_stage(prev))

    nc.compile()
    return nc


def _get_program():
    if "nc" not in _CACHE:
        _CACHE["nc"] = build_program()
    return _CACHE["nc"]


def make_cblobs(w_proj, b_proj):
    w = np.asarray(w_proj, dtype=np.float32)
    b = np.asarray(b_proj, dtype=np.float32)
    ch = np.zeros((128, 532), dtype=np.float16)
    sel = np.repeat(np.eye(4, dtype=np.float16), 128, axis=1)
    ch[0:4, 0:512] = sel
    for i in range(4):
        for o in range(4):
            ch[:, 512 + 4 * i + o] = np.float16(w[o, i])
    ch[0:4, 528:532] = 1.0
    cb = np.zeros((128, 1), dtype=np.float32)
    cb[0:4, 0] = b
    return ch, cb


LAST_RESULT = None


def kernel(g0, g1, g2, g3, w_proj, b_proj):
    global LAST_RESULT
    nc = _get_program()

    ch, cb = make_cblobs(w_proj, b_proj)

    gall = np.stack(
        [np.asarray(x).reshape(B, C, HW).astype(np.float16) for x in (g0, g1, g2, g3)],
        axis=1,
    )  # (B, 4, C, HW) fp16
    in_maps = []
    for bi in range(NCORES):
        m = {"gall": np.ascontiguousarray(gall[bi]), "cblob16": ch, "cblob": cb}
        in_maps.append(m)

    res = run_bass_kernel_spmd(
        nc,
        in_maps,
        list(range(NCORES)),
        trace=bool(int(os.environ.get("CM_TRACE", "0"))),
        tmpdir=os.environ.get("CM_TRACE_DIR") or None,
    )
    LAST_RESULT = res
    out_full = np.stack(
        [
            res.results[bi]["out"].astype(np.float32).reshape(C, H, W)
            for bi in range(NCORES)
        ],
        axis=0,
    )
    return out_full


# revision 22
# speedup vs baseline: 1.2039x; 1.1808x over previous
"""CrossMerge kernel for trn2 — v7 (fp16, PE broadcast, DVE chunk-folding).

Math (per batch element):
    means_i = mean over C of g_i              (4, H, W)
    logits  = w_proj @ means + b_proj         (4, H, W)
    w       = softmax(logits, axis=0)         (4, H, W)
    out     = sum_i g_i * w_i                 (C, H, W)

Sharding: data-parallel over batch B=8 across 8 cores; weights replicated;
no cross-device communication.

Measured history: v3 fp32 247us; v4 fp16 157.5us (PE-bound 148us busy);
v5b/v6 gpsimd-broadcast variants 182-186us — the gpsimd fp32 broadcast
writes 19-38MB through the SBUF ports and stalls concurrent DVE ops to
3.1us/op (vs their 952ns median), so broadcasts stay on PE+PSUM (PSUM has
its own ports; the ACT staging writes only the final 9.4MB of fp16).

v7 design:
 - Grids fp16 on HOST (HBM 23.7MB/core); output fp16, host upconverts.
 - DVE merge in fp16 2x_1p mode (0.52 ns/col, HW-verified): products as
   4 chunk-paired ops [128,2,1536] (wq broadcast over the chunk axis via
   an explicit 0-stride AP dim) + 3 paired adds, halving op count.
 - PE cost is passes x cols x 0.85ns (observed throttled clock) + 93ns
   LDWEIGHTS per matmul.  v4 ran 13 column passes on PE; v7 runs 10:
   grids 0-2's C-chunks are pre-folded on DVE (t_i = g_c0 + g_c1, fp16
   2x), so logits need 5 accumulating matmuls per jslice instead of 8.
   Folding all 4 would tip DVE past PE; 3 balances the two engines.
 - Broadcast staging via [128,768] PSUM tiles: per grid per 768-block,
   two matmuls (N=512+256, each within one PSUM bank) + ONE wide ACT
   copy PSUM->SBUF fp16 (halves v4's ACT op count and sem load).
 - Narrow tail per jslice: exp (ACT, scale=1/C exp-trick, bias=b_proj),
   S4 denominator (PE, ones lhsT), reciprocal_approx_fast (DVE, fp32),
   W4 = E*R4 -> fp16 into a d-tile-wide W4d (next iter's bcast rhs).
 - Per-iter emission (engine queue order is what matters):
     dma_in(d+1) | DVE folds(d) | PE bcast(d-1) + ACT staging | DVE
     products/adds(d-1) + dma_out(d-1) | narrow(d) | (DVE recip/W4 last)
   Folds go first on DVE so PE's logits(d) unblock early; products(d-1)
   keep DVE busy while the narrow(d) PE->ACT->PE chain round-trips.
"""

import os
import sys

import numpy as np

try:
    import concourse.bass as bass
except ImportError:  # fresh grading dir: concourse lives in the container repo
    sys.path.insert(0, "/opt/trn_rl_repo")
    import concourse.bass as bass

from contextlib import ExitStack

import concourse.tile as tile
from concourse import bacc, mybir
from concourse.bass_utils import run_bass_kernel_spmd

B, C, H, W = 8, 256, 96, 96
HW = H * W  # 9216
NCORES = 8
CPB = C // 128  # 2 partition chunks per core
MAXW = 1536  # d-tile width; 6 uniform tiles of 3 jslices
ND = HW // MAXW  # 6
NJ = MAXW // 512  # 3
NFOLD = 3  # grids 0..NFOLD-1 chunk-folded on DVE for the logits pass
BLK = 768  # bcast staging block (2 matmuls + 1 ACT copy per grid)

F32 = mybir.dt.float32
F16 = mybir.dt.float16
AF = mybir.ActivationFunctionType

_CACHE = {}


def bcast2(ap, n):
    """AP broadcast over a new middle 'chunk' axis of size n (stride 0)."""
    return bass.AP(ap.tensor, ap.offset, [ap.ap[0], [0, n], ap.ap[-1]])


def build_program():
    nc = bacc.Bacc("TRN2", debug=False, num_devices=NCORES)

    gall_d = nc.dram_tensor("gall", [4, C, HW], F16, kind="ExternalInput").ap()
    # fp16 constants: cols 0-511 sel (one-hot bcast lhsT rows 0-3), cols
    # 512-527 ws (w[o,i] at col 512+4i+o, replicated down partitions),
    # cols 528-531 ones4x4
    ch_d = nc.dram_tensor("cblob16", [128, 532], F16, kind="ExternalInput").ap()
    # fp32 constants: col 0 rows 0-3 = b_proj
    cb_d = nc.dram_tensor("cblob", [128, 1], F32, kind="ExternalInput").ap()
    out = nc.dram_tensor("out", [C, HW], F16, kind="ExternalOutput").ap()

    with tile.TileContext(nc) as tc, ExitStack() as ctx:
        const = ctx.enter_context(tc.tile_pool(name="const", bufs=1))
        gin = ctx.enter_context(tc.tile_pool(name="gin", bufs=3))
        outp = ctx.enter_context(tc.tile_pool(name="outp", bufs=2))
        foldp = ctx.enter_context(tc.tile_pool(name="foldp", bufs=2))
        narrow = ctx.enter_context(tc.tile_pool(name="narrow", bufs=3))
        wqp = ctx.enter_context(tc.tile_pool(name="wqp", bufs=2))
        prod = ctx.enter_context(tc.tile_pool(name="prod", bufs=3))
        qpool = ctx.enter_context(tc.tile_pool(name="qpool", bufs=3))
        ps_nar = ctx.enter_context(tc.tile_pool(name="psnar", bufs=2, space="PSUM"))
        ps_wb = ctx.enter_context(tc.tile_pool(name="pswb", bufs=2, space="PSUM"))

        ch = const.tile([128, 532], F16)
        nc.sync.dma_start(out=ch[:], in_=ch_d)
        cb = const.tile([128, 1], F32)
        nc.sync.dma_start(out=cb[:], in_=cb_d)
        bv = cb[0:4, 0:1]

        def ws_i(i):  # [128, 4] logits lhsT for grid i
            return ch[:, 512 + 4 * i : 512 + 4 * i + 4]

        def sel_i(i):  # [4, 128] bcast lhsT for grid i
            return ch[0:4, 128 * i : 128 * (i + 1)]

        ones4 = ch[0:4, 528:532]

        # Warmup matmul: absorbs the const-blob DMA wait on the PE clock.
        warm = ps_nar.tile([128, 512], F32, tag="smx")
        nc.tensor.matmul(warm[0:4, 0:16], lhsT=ch[0:4, 0:4], rhs=ch[0:4, 0:16],
                         start=True, stop=True)

        def fold_stage(gat):
            """DVE chunk-fold for grids 0..NFOLD-1 (fp16 2x): the logits
            contraction over (grid, chunk) shrinks from 8 to 5 matmuls."""
            ts = []
            for i in range(NFOLD):
                t = foldp.tile([128, MAXW], F16, tag=f"t{i}")
                nc.vector.tensor_add(t[:], gat[:, i, 0, :], gat[:, i, 1, :])
                ts.append(t)
            return ts

        def narrow_stage(d, gat, ts):
            """Softmax chain: logits (jslice pairs) -> exp -> S4 -> recip
            -> W4 (fp16, d-tile-wide for next iter's bcast rhs).  Matmul
            outputs at PSUM base partition 0 (ISA constraint)."""
            jslc = [(x0, 512) for x0 in range(0, MAXW, 512)]
            W4d = narrow.tile([4, MAXW], F16, tag="W4", bufs=2)
            for pair in [jslc[k : k + 2] for k in range(0, len(jslc), 2)]:
                Ls, Es = [], []
                for x0, n in pair:
                    L = ps_nar.tile([128, 512], F32, tag="smx")
                    Ls.append(L[0:4, 0:n])
                    rhss = [ts[i][:, x0 : x0 + n] for i in range(NFOLD)]
                    rhss += [
                        gat[:, i, c, x0 : x0 + n]
                        for i in range(NFOLD, 4)
                        for c in range(CPB)
                    ]
                    lhss = [ws_i(i) for i in range(NFOLD)] + [
                        ws_i(i) for i in range(NFOLD, 4) for _ in range(CPB)
                    ]
                    for k, (lh, rh) in enumerate(zip(lhss, rhss)):
                        nc.tensor.matmul(Ls[-1], lhsT=lh, rhs=rh,
                                         start=(k == 0),
                                         stop=(k == len(rhss) - 1))
                for pi, (x0, n) in enumerate(pair):
                    E = narrow.tile([4, 512], F16, tag="E")
                    nc.scalar.activation(E[0:4, 0:n], Ls[pi], AF.Exp,
                                         bias=bv, scale=1.0 / C)
                    Es.append(E[0:4, 0:n])
                S4s = []
                for pi, (x0, n) in enumerate(pair):
                    S4 = ps_nar.tile([4, 512], F32, tag="S4")
                    nc.tensor.matmul(S4[0:4, 0:n], lhsT=ones4, rhs=Es[pi],
                                     start=True, stop=True)
                    S4s.append(S4[0:4, 0:n])
                for pi, (x0, n) in enumerate(pair):
                    R4 = narrow.tile([4, 512], F32, tag="R4", bufs=2)
                    nc.vector.reciprocal_approx_fast(R4[0:4, 0:n], S4s[pi])
                    nc.vector.tensor_mul(W4d[0:4, x0 : x0 + n], Es[pi],
                                         R4[0:4, 0:n])
            return W4d

        def bcast_stage(prev):
            """PE broadcast of W4 rows to 128 partitions, staged to fp16
            SBUF by wide [128,768] ACT copies (each matmul writes within a
            single PSUM bank; the copy spans banks, reads are unrestricted)."""
            if prev is None:
                return None
            d, gat, ot, W4d = prev
            wq = {}
            for i in range(4):
                wqt = wqp.tile([128, MAXW], F16, tag=f"wq{i}")
                wq[i] = wqt
            for b0 in range(0, MAXW, BLK):
                for i in range(4):
                    Wb = ps_wb.tile([128, BLK], F32, tag="wb")
                    for s0 in range(0, BLK, 512):
                        n = min(512, BLK - s0)
                        nc.tensor.matmul(
                            Wb[:, s0 : s0 + n],
                            lhsT=sel_i(i),
                            rhs=W4d[0:4, b0 + s0 : b0 + s0 + n],
                            start=True, stop=True,
                        )
                    nc.scalar.copy(wq[i][:, b0 : b0 + BLK], Wb[:])
            return (d, gat, ot, wq)

        def wide_stage(staged):
            """DVE products + add tree (chunk-paired ops) + store."""
            if staged is None:
                return
            d, gat, ot, wq = staged
            p = {}
            for i in range(4):
                pt = prod.tile([128, CPB, MAXW], F16, tag="p")
                nc.vector.tensor_mul(pt[:], gat[:, i, :, :],
                                     bcast2(wq[i][:], CPB))
                p[i] = pt
                if i == 1:
                    q01 = qpool.tile([128, CPB, MAXW], F16, tag="q")
                    nc.vector.tensor_add(q01[:], p[0][:], p[1][:])
            q23 = qpool.tile([128, CPB, MAXW], F16, tag="q")
            nc.vector.tensor_add(q23[:], p[2][:], p[3][:])
            nc.vector.tensor_add(ot[:], q01[:], q23[:])
            n0 = d * MAXW
            nc.sync.dma_start(
                out=out[:, n0 : n0 + MAXW].rearrange("(c p) n -> p c n", c=CPB),
                in_=ot[:],
            )

        def dma_in(d):
            n0 = d * MAXW
            gat = gin.tile([128, 4, CPB, MAXW], F16, tag="gall")
            nc.sync.dma_start(
                out=gat[:],
                in_=gall_d[:, :, n0 : n0 + MAXW].rearrange(
                    "i (c p) n -> p i c n", c=CPB
                ),
            )
            return gat

        gats = {0: dma_in(0)}
        prev = None  # (d, gat, ot, W4d) awaiting bcast+wide
        for d in range(ND):
            if d + 1 < ND:
                gats[d + 1] = dma_in(d + 1)
            gat = gats.pop(d)
            ts = fold_stage(gat)
            staged = bcast_stage(prev)
            wide_stage(staged)
            ot = outp.tile([128, CPB, MAXW], F16, tag="ot")
            W4d = narrow_stage(d, gat, ts)
            prev = (d, gat, ot, W4d)
        wide_stage(bcast_stage(prev))

    nc.compile()
    return nc


def _get_program():
    if "nc" not in _CACHE:
        _CACHE["nc"] = build_program()
    return _CACHE["nc"]


def make_cblobs(w_proj, b_proj):
    w = np.asarray(w_proj, dtype=np.float32)
    b = np.asarray(b_proj, dtype=np.float32)
    ch = np.zeros((128, 532), dtype=np.float16)
    sel = np.repeat(np.eye(4, dtype=np.float16), 128, axis=1)
    ch[0:4, 0:512] = sel
    for i in range(4):
        for o in range(4):
            ch[:, 512 + 4 * i + o] = np.float16(w[o, i])
    ch[0:4, 528:532] = 1.0
    cb = np.zeros((128, 1), dtype=np.float32)
    cb[0:4, 0] = b
    return ch, cb


LAST_RESULT = None


def kernel(g0, g1, g2, g3, w_proj, b_proj):
    global LAST_RESULT
    nc = _get_program()

    ch, cb = make_cblobs(w_proj, b_proj)

    gall = np.stack(
        [np.asarray(x).reshape(B, C, HW).astype(np.float16) for x in (g0, g1, g2, g3)],
        axis=1,
    )  # (B, 4, C, HW) fp16
    in_maps = []
    for bi in range(NCORES):
        m = {"gall": np.ascontiguousarray(gall[bi]), "cblob16": ch, "cblob": cb}
        in_maps.append(m)

    res = run_bass_kernel_spmd(
        nc,
        in_maps,
        list(range(NCORES)),
        trace=bool(int(os.environ.get("CM_TRACE", "0"))),
        tmpdir=os.environ.get("CM_TRACE_DIR") or None,
    )
    LAST_RESULT = res
    out_full = np.stack(
        [
            res.results[bi]["out"].astype(np.float32).reshape(C, H, W)
            for bi in range(NCORES)
        ],
        axis=0,
    )
    return out_full
